# revision 1
# baseline (speedup 1.0000x reference)
"""Bilateral filter (nn_BilateralFilter) Trainium2 Bass kernel.

Reference semantics (KERNEL_SIZE=5, THETA_ALPHA=2.0, THETA_BETA=0.1):
    w_k   = exp(-(dx^2+dy^2)/8)                      (24 offsets, center dropped)
    Ki    = exp(-50*(I(p+k) - I(p))^2)               per image channel c
    out[c,n,p] = sum_k w_k*Ki[c,k,p]*Q(n,p+k) / sum_k w_k*Ki[c,k,p]

Sharding: 8 cores = 2 batches x 4 row-slabs of 80 output rows, each slab
shipped with a 2-row halo and 2-col zero padding (84 x 324 per channel).
Device layout: partitions = image rows, free dim = (channel, column).
fp16 on-chip (DVE 2x mode); exponent computed via ACT (Square then in-place
Exp with the spatial weight folded into the per-slot exp bias, plus a +8
exponent shift that keeps fp16 sums well inside normal range and cancels in
the final division).  Compute-engine SBUF accesses spanning >32 partitions
must start at partition 0, so each row shift dr gets its own 80-partition
copy (5 blocks packed in one tile, per-dr DMAs for early start).  Products
are batched over all 5 dc and broadcast over the 6 classes in one
4-dim-AP DVE op per (channel, dr); sums use flat pairwise folds.
Measured: ~199 us on HW (max core), L2 rel err ~6.4e-4 vs the fp32 reference.
"""

import math

import numpy as np

B, C, NCL = 2, 3, 6
H = W = 320
KS, PAD = 5, 2
NK = KS * KS - 1          # 24
WP = W + 2 * PAD          # 324
NSLAB = 4
R = H // NSLAB            # 80 output rows per shard
RH = R + 2 * PAD          # 84 rows incl. halo
COEF = 50.0               # 1/(2*theta_beta^2)
SHIFT = 8.0               # exponent shift, cancels in the division
IW = C * WP               # 972
QW = NCL * WP             # 1944

_CACHE: dict = {}


def _offsets():
    return [
        (dr, dc)
        for dr in range(KS)
        for dc in range(KS)
        if not (dr == PAD and dc == PAD)
    ]


def _emit(tc, i_ap, q_ap, out_ap):
    """Emit the per-core program into TileContext tc.

    i_ap:   DRAM AP (RH, C*WP)  fp16
    q_ap:   DRAM AP (RH, NCL*WP) fp16
    out_ap: DRAM AP (R, C*NCL*W) fp16

    Layout: 25 k-slots (dr-major, center included but killed via a -30
    exp bias so it contributes exactly 0), each slot holding (c, x).
    Products are batched over all 5 dc per (c, dr) in one 4-dim-AP op.
    """
    import concourse.bass as bass
    import concourse.mybir as mybir

    f16 = mybir.dt.float16
    f32 = mybir.dt.float32
    AF = mybir.ActivationFunctionType
    nc = tc.nc
    NS = KS * KS           # 25 slots
    CW = C * W             # 960, slot width in d/sq/kw tiles
    CTR = PAD * KS + PAD   # slot 12 = center

    with tc.tile_pool(name="p", bufs=1) as pool:
        # exp biases as per-partition const columns (activation bias AP)
        bias_vals = {}
        for dr in range(KS):
            for dc in range(KS):
                s = dr * KS + dc
                if s == CTR:
                    bias_vals[s] = SHIFT - 30.0
                else:
                    bias_vals[s] = (
                        SHIFT - ((dr - PAD) ** 2 + (dc - PAD) ** 2) / 8.0
                    )
        distinct = sorted(set(bias_vals.values()))
        bcol = {v: j for j, v in enumerate(distinct)}
        bias_t = pool.tile([R, len(distinct)], f32, tag="bias")
        for v, j in bcol.items():
            nc.vector.memset(bias_t[:, j : j + 1], v)

        # dr processing order: center block first (every sub reads it)
        DRS = [2, 0, 1, 3, 4]

        Ia = pool.tile([R, KS * IW], f16, tag="Ia")
        Qa = pool.tile([R, KS * QW], f16, tag="Qa")
        # per-dr-block DMAs so the first subs/products start early
        for dr in DRS:
            nc.sync.dma_start(
                Ia[:, dr * IW : (dr + 1) * IW], i_ap[dr : dr + R, :]
            )
        for dr in DRS:
            # issue Q loads from the ACT sequencer's DGE (idle at start) so
            # their traffic doesn't delay the I-block completion sems
            nc.scalar.dma_start(
                Qa[:, dr * QW : (dr + 1) * QW], q_ap[dr : dr + R, :]
            )

        def i_v(dr, dc):
            # [R, (c,320)] view of I at offset (dr, dc); c-stride WP
            return bass.AP(
                tensor=Ia.tensor, offset=Ia.offset + dr * IW + dc,
                ap=[[KS * IW, R], [WP, C], [1, W]],
            )

        # d[slot] = I(p+k) - I(p), all 3 channels per op; ACT square + exp
        # interleaved per dr so the first products unblock early
        d = pool.tile([R, NS * CW], f16, tag="big0")
        sq = pool.tile([R, NS * CW], f16, tag="big1")
        kw = sq  # exp runs in place over sq (elementwise ACT stream)
        for dr in DRS:
            # one 4-dim-AP sub for all 5 dc x 3 c; center slot yields 0
            lo, hi = dr * KS * CW, (dr + 1) * KS * CW
            dst = d[:, lo:hi].rearrange("p (dc c w) -> p dc c w", dc=KS, c=C)
            in0 = bass.AP(
                tensor=Ia.tensor, offset=Ia.offset + dr * IW,
                ap=[[KS * IW, R], [1, KS], [WP, C], [1, W]],
            )
            in1 = bass.AP(
                tensor=Ia.tensor, offset=Ia.offset + PAD * IW + PAD,
                ap=[[KS * IW, R], [0, KS], [WP, C], [1, W]],
            )
            nc.vector.tensor_sub(dst, in0, in1)
            nc.scalar.activation(sq[:, lo:hi], d[:, lo:hi], AF.Square)
            for dc in range(KS):
                s = dr * KS + dc
                j = bcol[bias_vals[s]]
                nc.scalar.activation(
                    kw[:, s * CW : (s + 1) * CW],
                    sq[:, s * CW : (s + 1) * CW],
                    AF.Exp,
                    bias=bias_t[:, j : j + 1],
                    scale=-COEF,
                )

        # Products dr-outer / c-inner: each exp group (one dr) feeds ~3x
        # more DVE work, so DVE never stalls on the ACT exp pipeline.
        NW = NCL * W
        accs = [pool.tile([R, NCL * W], f16, tag=f"acc{c}", name=f"acc{c}") for c in range(C)]
        for dr in DRS:
            for c in range(C):
                acc = accs[c]
                # P5[dc, n, x] = kw[5dr+dc, c, x] * Q[n, p+(dr,dc-2)]
                P5 = pool.tile([R, KS * NCL * W], f16, tag="P5", bufs=1)
                if dr == PAD:
                    # center slot is zero - compute only taps {10,11},{13,14}
                    for h, (s0, qo) in enumerate(((10, 0), (13, 3))):
                        kw_src = bass.AP(
                            tensor=kw.tensor,
                            offset=kw.offset + s0 * CW + c * W,
                            ap=[[NS * CW, R], [CW, 2], [0, NCL], [1, W]],
                        )
                        q_src = bass.AP(
                            tensor=Qa.tensor,
                            offset=Qa.offset + dr * QW + qo,
                            ap=[[KS * QW, R], [1, 2], [WP, NCL], [1, W]],
                        )
                        nc.vector.tensor_mul(
                            P5[:, h * 2 * NW : (h + 1) * 2 * NW].rearrange(
                                "p (dc n w) -> p dc n w", dc=2, n=NCL
                            ),
                            kw_src,
                            q_src,
                        )
                    s1 = pool.tile([R, 2 * NW], f16, tag="s1", bufs=1)
                    nc.vector.tensor_add(
                        s1[:, :], P5[:, : 2 * NW], P5[:, 2 * NW : 4 * NW]
                    )
                    # dr=PAD is first in DRS: fold straight into acc
                    nc.vector.tensor_add(acc[:, :], s1[:, :NW], s1[:, NW:])
                    continue
                kw_src = bass.AP(
                    tensor=kw.tensor,
                    offset=kw.offset + (dr * KS) * CW + c * W,
                    ap=[[NS * CW, R], [CW, KS], [0, NCL], [1, W]],
                )
                q_src = bass.AP(
                    tensor=Qa.tensor, offset=Qa.offset + dr * QW,
                    ap=[[KS * QW, R], [1, KS], [WP, NCL], [1, W]],
                )
                nc.vector.tensor_mul(
                    P5[:, :].rearrange("p (dc n w) -> p dc n w", dc=KS, n=NCL),
                    kw_src,
                    q_src,
                )
                # fold 5 -> 1: [A+C, B+D] ; + ; + E
                s1 = pool.tile([R, 2 * NW], f16, tag="s1", bufs=1)
                nc.vector.tensor_add(
                    s1[:, :], P5[:, : 2 * NW], P5[:, 2 * NW : 4 * NW]
                )
                s2 = pool.tile([R, NW], f16, tag="s2", bufs=1)
                nc.vector.tensor_add(s2[:, :], s1[:, :NW], s1[:, NW:])
                s3 = pool.tile([R, NW], f16, tag="s1", bufs=1)
                nc.vector.tensor_add(s3[:, :], s2[:, :], P5[:, 4 * NW :])
                nc.vector.tensor_add(acc[:, :], acc[:, :], s3[:, :])

        # norm for all channels at once, in kw's native [slot,(c,x)] layout:
        # flat pairwise folds 24 -> 12 -> 6 -> 3 -> 1 (+ zero center slot).
        nt_a = pool.tile([R, 12 * CW], f16, tag="big0")
        nc.vector.tensor_add(
            nt_a[:, :], kw[:, : 12 * CW], kw[:, 12 * CW : 24 * CW]
        )
        nt_b = pool.tile([R, 6 * CW], f16, tag="nt_b")
        nc.vector.tensor_add(nt_b[:, :], nt_a[:, : 6 * CW], nt_a[:, 6 * CW :])
        nt_c = pool.tile([R, 3 * CW], f16, tag="nt_c")
        nc.vector.tensor_add(nt_c[:, :], nt_b[:, : 3 * CW], nt_b[:, 3 * CW :])
        n1 = pool.tile([R, CW], f16, tag="n1")
        nc.vector.tensor_add(n1[:, :], nt_c[:, :CW], nt_c[:, CW : 2 * CW])
        nc.vector.tensor_add(n1[:, :], n1[:, :], nt_c[:, 2 * CW : 3 * CW])
        norm = pool.tile([R, CW], f32, tag="norm")
        nc.vector.tensor_add(norm[:, :], n1[:, :], kw[:, 24 * CW : 25 * CW])
        rnorm = pool.tile([R, CW], f32, tag="rnorm")
        nc.vector.reciprocal_approx_fast(rnorm[:, :], norm[:, :])
        rnh = pool.tile([R, CW], f16, tag="rnh")
        nc.vector.tensor_copy(rnh[:, :], rnorm[:, :])

        for c in range(C):
            acc = accs[c]
            ot = pool.tile([R, NCL * W], f16, tag="out", bufs=2)
            rb = (
                rnh[:, c * W : (c + 1) * W]
                .unsqueeze(1)
                .broadcast_to([R, NCL, W])
            )
            nc.vector.tensor_mul(
                ot[:, :].rearrange("p (n w) -> p n w", n=NCL),
                acc[:, :].rearrange("p (n w) -> p n w", n=NCL),
                rb,
            )
            nc.sync.dma_start(
                out_ap[:, c * NCL * W : (c + 1) * NCL * W], ot[:, :]
            )


def _build_program():
    import concourse.bacc as bacc
    import concourse.mybir as mybir
    from concourse import tile

    f16 = mybir.dt.float16

    nc = bacc.Bacc("TRN2", num_devices=8, debug=False)
    I_in = nc.dram_tensor("i_in", [RH, IW], f16, kind="ExternalInput")
    Q_in = nc.dram_tensor("q_in", [RH, QW], f16, kind="ExternalInput")
    OUT = nc.dram_tensor("out", [R, C * NCL * W], f16, kind="ExternalOutput")

    with tile.TileContext(nc) as tc:
        _emit(tc, I_in.ap(), Q_in.ap(), OUT.ap())

    nc.compile()
    return nc


def _get_program():
    if "nc" not in _CACHE:
        _CACHE["nc"] = _build_program()
    return _CACHE["nc"]


def _shard_inputs(Q, I):
    """Host prep: pad, cast fp16, per-shard (rows, chan*cols) layout."""
    Qp = np.pad(
        np.asarray(Q, np.float32), ((0, 0), (0, 0), (PAD, PAD), (PAD, PAD))
    ).astype(np.float16)
    Ip = np.pad(
        np.asarray(I, np.float32), ((0, 0), (0, 0), (PAD, PAD), (PAD, PAD))
    ).astype(np.float16)
    in_maps = []
    for b in range(B):
        for s in range(NSLAB):
            r0 = s * R
            i_sh = Ip[b, :, r0 : r0 + RH, :]  # (C, RH, WP)
            q_sh = Qp[b, :, r0 : r0 + RH, :]  # (NCL, RH, WP)
            in_maps.append(
                {
                    "i_in": np.ascontiguousarray(
                        i_sh.transpose(1, 0, 2).reshape(RH, IW)
                    ),
                    "q_in": np.ascontiguousarray(
                        q_sh.transpose(1, 0, 2).reshape(RH, QW)
                    ),
                }
            )
    return in_maps


def _assemble(outs):
    # outs: list of 8 arrays (R, C*NCL*W), core order = (b, slab)
    o = np.stack([np.asarray(x) for x in outs]).astype(np.float32)
    o = o.reshape(B, NSLAB, R, C, NCL, W)
    o = o.transpose(0, 3, 4, 1, 2, 5).reshape(B, C, NCL, H, W)
    return o


def run(Q, I, trace=False):
    from concourse.bass_utils import run_bass_kernel_spmd

    nc = _get_program()
    in_maps = _shard_inputs(Q, I)
    res = run_bass_kernel_spmd(nc, in_maps, list(range(8)), trace=trace)
    out = _assemble([res.results[i]["out"] for i in range(8)])
    return out, res


def kernel(Q, I):
    out, _ = run(Q, I)
    return out



# revision 5
# speedup vs baseline: 1.7384x; 1.7384x over previous
"""Bilateral filter (nn_BilateralFilter) Trainium2 Bass kernel, v2.

Reference semantics (KERNEL_SIZE=5, THETA_ALPHA=2.0, THETA_BETA=0.1):
    w_k   = exp(-(dx^2+dy^2)/8)                      (24 offsets, center dropped)
    Ki    = exp(-50*(I(p+k) - I(p))^2)               per image channel c
    out[c,n,p] = sum_k w_k*Ki[c,k,p]*Q(n,p+k) / sum_k w_k*Ki[c,k,p]

v2 strategy (vs v1's all-DVE fold at 199us):
  The range kernel is symmetric in its two endpoints, so
  Ki_k(p) = Ki_{-k}(p+k) and therefore
      Qtilde(p) = sum_j U_j(p - j),   U_j(q) = w_j*Ki_j(q)*Q(q)
  i.e. every product is SAME-SITE (no shifted Q reads), and the 24-tap
  shifted fold becomes 24 one-hot-shift matmuls accumulated in fp32 PSUM
  on the otherwise-idle TensorEngine.  Per core: partitions = (c=3, y=42)
  over two U-row groups [-2,40) and [40,82); row shifts live in the
  host-precomputed 0/1 stationaries (with 2-row cross-group halo
  matmuls), column shifts are free-dim offsets of the moving operand.
  DVE keeps only subs, the 24 U-products (fp16, 2x mode), reciprocal and
  the final PSUM*1/norm; ACT does Square and per-slot Exp with the full
  spatial weight folded into the exp bias (+SHIFT trick cancels in the
  division).  norm(p) = sum_k w_k Ki_k(p) rides the same PSUM path with
  unshifted stationaries.
Sharding: 8 cores = 2 batches x 4 row-slabs of 80 output rows.
"""

import numpy as np

B, C, NCL = 2, 3, 6
H = W = 320
KS, PAD = 5, 2
NSLAB = 4
R = H // NSLAB            # 80 output rows per shard
COEF = 50.0               # 1/(2*theta_beta^2)
SHIFT = 8.0               # exponent shift, cancels in the division
YG = 42                   # U-rows per group ([-2,40) and [40,82))
NP_ = 3 * YG              # 126 partitions for U/kw tiles
MP = 3 * 40               # 120 output partitions (c, y40)
XI = W + 8                # 328: I cols incl +-4 pad
XU = W + 4                # 324: U/kw cols incl +-2 pad
IR = R + 8                # 88 I rows per core
QR = R + 4                # 84 Q rows per core
NST = 14                  # stationaries

_CACHE: dict = {}


def _stationaries():
    """14 one-hot stationaries [126, 14*120] fp16, built on host.

    Slot map (dr = raw 0..4, dr' = dr-2):
      0..4  : S_A_dr   main grp A: p=(c, py=y+2-dr'), m=(c,y), py<42
      5..6  : S_Ah_dr  halo  grp A (dr=0,1): p=(c, pyB), U^B rows 40,41
      7..11 : S_B_dr   main grp B: p=(c, py=my-dr'),  m=(c,my), py>=0
      12..13: S_Bh_dr  halo  grp B (dr=3,4): p=(c, pyA), U^A rows 38,39
    """
    s = np.zeros((NP_, NST * MP), np.float32)

    def S(idx):
        return s[:, idx * MP:(idx + 1) * MP].reshape(NP_, 3, 40)

    for dr in range(5):
        drp = dr - 2
        sa = S(dr)
        sb = S(7 + dr)
        for c in range(3):
            for y in range(40):
                py = y + 2 - drp
                if 0 <= py < YG:
                    sa[c * YG + py, c, y] = 1.0
                py = y - drp
                if 0 <= py < YG:
                    sb[c * YG + py, c, y] = 1.0
    # grp A halo: U rows q=40,41 live in tile B partitions 0,1
    for i, dr in enumerate((0, 1)):
        sh = S(5 + i)
        for c in range(3):
            for pyb in range(2):
                y = 40 + pyb + dr - 2
                if 0 <= y < 40:
                    sh[c * YG + pyb, c, y] = 1.0
    # grp B halo: U rows q=38,39 live in tile A partitions 40,41
    for i, dr in enumerate((3, 4)):
        sh = S(12 + i)
        for c in range(3):
            for pya in (40, 41):
                q = pya - 2
                my = q + (dr - 2) - 40
                if 0 <= my < 40:
                    sh[c * YG + pya, c, my] = 1.0
    return s.astype(np.float16)


def _emit(tc, i_ap, q_ap, s_ap, out_ap):
    """i_ap: (264, 328) fp16   rows = (c, 88)
    q_ap: (84, 1944) fp16     rows = U-rows [-2,82), cols (n,324)
    s_ap: (126, 1680) fp16    stationaries
    out_ap: (80, 5760) fp16   cols (c, n, x)
    """
    import concourse.bass as bass
    import concourse.mybir as mybir

    f16 = mybir.dt.float16
    f32 = mybir.dt.float32
    AF = mybir.ActivationFunctionType
    nc = tc.nc
    SLOTS = [(dr, dc) for dr in range(5) for dc in range(5)
             if not (dr == 2 and dc == 2)]
    BIAS = {
        (dr, dc): SHIFT - ((dr - 2) ** 2 + (dc - 2) ** 2) / 8.0
        for dr, dc in SLOTS
    }

    with tc.tile_pool(name="p", bufs=1) as pool, \
            tc.psum_pool(name="pp", bufs=1) as ppool:
        stat = pool.tile([NP_, NST * MP], f16, tag="stat")
        nc.sync.dma_start(stat[:, :], s_ap[:, :])

        distinct = sorted(set(BIAS.values()))
        bcol = {v: j for j, v in enumerate(distinct)}
        bias_t = pool.tile([NP_, len(distinct)], mybir.dt.float32, tag="bias")
        for v, j in bcol.items():
            nc.vector.memset(bias_t[:, j:j + 1], v)

        def st(idx):
            return stat[:, idx * MP:(idx + 1) * MP]

        # per-(grp, dr) shifted I copies [(c,42), 328]
        Idr = {}
        for g in range(2):
            for dr in range(5):
                t = pool.tile([NP_, XI], f16, tag=f"I{g}{dr}")
                for c in range(3):
                    nc.sync.dma_start(
                        t[c * YG:(c + 1) * YG, :],
                        i_ap[c * IR + g * YG + dr: c * IR + g * YG + dr + YG, :],
                    )
                Idr[(g, dr)] = t
        # Q replicated over c: [(c,42), (n,324)] per grp
        Qrep = []
        for g in range(2):
            t = pool.tile([NP_, NCL * XU], f16, tag=f"Q{g}")
            for c in range(3):
                nc.scalar.dma_start(
                    t[c * YG:(c + 1) * YG, :],
                    q_ap[g * YG:(g + 1) * YG, :],
                )
            Qrep.append(t)

        # kw[(c,42), (slot25, 324)]: d -> square -> exp(in place)
        kw = []
        for g in range(2):
            kwg = pool.tile([NP_, 25 * XU], f16, tag=f"kw{g}")
            d = pool.tile([NP_, 5 * XU], f16, tag=f"d{g}", bufs=2)
            for dr in range(5):
                # d[dc, x] = I(q + (dr', dc')) - I(q); all 5 dc in one op
                in0 = bass.AP(
                    tensor=Idr[(g, dr)].tensor, offset=Idr[(g, dr)].offset,
                    ap=[[XI, NP_], [1, 5], [1, XU]],
                )
                in1 = bass.AP(
                    tensor=Idr[(g, 2)].tensor, offset=Idr[(g, 2)].offset + 2,
                    ap=[[XI, NP_], [0, 5], [1, XU]],
                )
                dst = d[:, :].rearrange("p (dc x) -> p dc x", dc=5)
                nc.vector.tensor_sub(dst, in0, in1)
                nc.scalar.activation(
                    kwg[:, dr * 5 * XU:(dr + 1) * 5 * XU], d[:, :], AF.Square
                )
            for dr, dc in SLOTS:
                sl = dr * 5 + dc
                nc.scalar.activation(
                    kwg[:, sl * XU:(sl + 1) * XU],
                    kwg[:, sl * XU:(sl + 1) * XU],
                    AF.Exp, bias=bias_t[:, bcol[BIAS[(dr, dc)]]:
                                        bcol[BIAS[(dr, dc)]] + 1],
                    scale=-COEF,
                )
            kw.append(kwg)

        # PSUM: per grp 4 fbanks of (n6, x80) + 1 norm bank
        NFB = 4
        XB = W // NFB  # 80
        qt = [ppool.tile([MP, NCL * XB], f32, tag=f"qt{b}", name=f"qt{b}")
              for b in range(NFB)]
        nrm = ppool.tile([MP, W], f32, tag="nrm", name="nrm")

        def u_prod(g, dr, dc, tag, bufs=1):
            sl = dr * 5 + dc
            u = pool.tile([NP_, NCL * XU], f16, tag=tag, bufs=bufs)
            kws = bass.AP(
                tensor=kw[g].tensor, offset=kw[g].offset + sl * XU,
                ap=[[25 * XU, NP_], [0, NCL], [1, XU]],
            )
            qs = bass.AP(
                tensor=Qrep[g].tensor, offset=Qrep[g].offset,
                ap=[[NCL * XU, NP_], [XU, NCL], [1, XU]],
            )
            nc.vector.tensor_mul(
                u[:, :].rearrange("p (n x) -> p n x", n=NCL), kws, qs
            )
            return u

        def mm_u(g, u, dr, dc, sidx, start, stop):
            for b in range(NFB):
                mv = bass.AP(
                    tensor=u.tensor, offset=u.offset + (4 - dc) + b * XB,
                    ap=[[NCL * XU, NP_], [XU, NCL], [1, XB]],
                )
                nc.tensor.matmul(
                    qt[b][:, :], st(sidx), mv,
                    start=start, stop=stop,
                )

        def mm_norm(g, dr, dc, start, stop):
            sl = dr * 5 + dc
            mv = bass.AP(
                tensor=kw[g].tensor, offset=kw[g].offset + sl * XU + 2,
                ap=[[25 * XU, NP_], [1, W]],
            )
            nc.tensor.matmul(
                nrm[:, :], st(2 if g == 0 else 9), mv,
                start=start, stop=stop,
            )

        # ---- grp A: norm MMs first (only need kw), then main ----
        for i, (dr, dc) in enumerate(SLOTS):
            mm_norm(0, dr, dc, start=i == 0, stop=i == len(SLOTS) - 1)
        upinA = {}
        first = True
        for dr in range(5):
            for dc in range(5):
                if (dr, dc) == (2, 2):
                    continue
                if dr >= 3:
                    u = u_prod(0, dr, dc, tag=f"up{dr}{dc}")
                    upinA[(dr, dc)] = u
                else:
                    u = u_prod(0, dr, dc, tag="urot", bufs=4)
                mm_u(0, u, dr, dc, dr, start=first, stop=False)
                first = False
        # ---- grp B products for dr 0,1 feed grp A halo ----
        uB01 = {}
        for dr in range(2):
            for dc in range(5):
                u = u_prod(1, dr, dc, tag=f"uB{dr}{dc}")
                uB01[(dr, dc)] = u
                last = (dr, dc) == (1, 4)
                mm_u(0, u, dr, dc, 5 + dr, start=False, stop=last)

        # ---- grp A finals: rnorm + PSUM*rnorm -> out ----
        def finals(g):
            rn = pool.tile([MP, W], f32, tag=f"rn{g}")
            nc.vector.reciprocal_approx_fast(rn[:, :], nrm[:, :])
            ot = pool.tile([MP, NCL * W], f16, tag=f"ot{g}")
            for b in range(NFB):
                dst = bass.AP(
                    tensor=ot.tensor, offset=ot.offset + b * XB,
                    ap=[[NCL * W, MP], [W, NCL], [1, XB]],
                )
                rnb = bass.AP(
                    tensor=rn.tensor, offset=rn.offset + b * XB,
                    ap=[[W, MP], [0, NCL], [1, XB]],
                )
                src = qt[b][:, :].rearrange("p (n x) -> p n x", n=NCL)
                nc.vector.tensor_mul(dst, src, rnb)
            for c in range(3):
                nc.sync.dma_start(
                    out_ap[g * 40:(g + 1) * 40,
                           c * NCL * W:(c + 1) * NCL * W],
                    ot[c * 40:(c + 1) * 40, :],
                )

        finals(0)

        # ---- grp B: norm MMs, then main; halo from pinned U^A ----
        for i, (dr, dc) in enumerate(SLOTS):
            mm_norm(1, dr, dc, start=i == 0, stop=i == len(SLOTS) - 1)
        first = True
        for dr in range(5):
            for dc in range(5):
                if (dr, dc) == (2, 2):
                    continue
                if dr < 2:
                    u = uB01[(dr, dc)]
                else:
                    u = u_prod(1, dr, dc, tag="urot", bufs=4)
                mm_u(1, u, dr, dc, 7 + dr, start=first, stop=False)
                first = False
        for i, dr in enumerate((3, 4)):
            for dc in range(5):
                last = (dr, dc) == (4, 4)
                mm_u(1, upinA[(dr, dc)], dr, dc, 12 + i,
                     start=False, stop=last)

        finals(1)


def _build_program():
    import concourse.bacc as bacc
    import concourse.mybir as mybir
    from concourse import tile

    f16 = mybir.dt.float16

    nc = bacc.Bacc("TRN2", num_devices=8, debug=False)
    I_in = nc.dram_tensor("i_in", [3 * IR, XI], f16, kind="ExternalInput")
    Q_in = nc.dram_tensor("q_in", [QR, NCL * XU], f16, kind="ExternalInput")
    S_in = nc.dram_tensor("s_in", [NP_, NST * MP], f16, kind="ExternalInput")
    OUT = nc.dram_tensor("out", [R, C * NCL * W], f16, kind="ExternalOutput")

    with tile.TileContext(nc) as tc:
        _emit(tc, I_in.ap(), Q_in.ap(), S_in.ap(), OUT.ap())

    nc.compile()
    return nc


def _get_program():
    if "nc" not in _CACHE:
        _CACHE["nc"] = _build_program()
    return _CACHE["nc"]


def _shard_inputs(Q, I):
    """Host prep: pad, cast fp16, per-shard layouts (see _emit)."""
    Qp = np.pad(
        np.asarray(Q, np.float32), ((0, 0), (0, 0), (PAD, PAD), (PAD, PAD))
    ).astype(np.float16)
    Ip = np.pad(
        np.asarray(I, np.float32), ((0, 0), (0, 0), (4, 4), (4, 4))
    ).astype(np.float16)
    s_np = _stationaries()
    in_maps = []
    for b in range(B):
        for s in range(NSLAB):
            r0 = s * R
            i_sh = Ip[b, :, r0:r0 + IR, :]            # (C, 88, 328)
            q_sh = Qp[b, :, r0:r0 + QR, :]            # (NCL, 84, 324)
            in_maps.append(
                {
                    "i_in": np.ascontiguousarray(i_sh.reshape(3 * IR, XI)),
                    "q_in": np.ascontiguousarray(
                        q_sh.transpose(1, 0, 2).reshape(QR, NCL * XU)
                    ),
                    "s_in": s_np,
                }
            )
    return in_maps


def _assemble(outs):
    # outs: list of 8 arrays (R, C*NCL*W), core order = (b, slab)
    o = np.stack([np.asarray(x) for x in outs]).astype(np.float32)
    o = o.reshape(B, NSLAB, R, C, NCL, W)
    o = o.transpose(0, 3, 4, 1, 2, 5).reshape(B, C, NCL, H, W)
    return o


def run(Q, I, trace=False):
    from concourse.bass_utils import run_bass_kernel_spmd

    nc = _get_program()
    in_maps = _shard_inputs(Q, I)
    res = run_bass_kernel_spmd(nc, in_maps, list(range(8)), trace=trace)
    out = _assemble([res.results[i]["out"] for i in range(8)])
    return out, res


def kernel(Q, I):
    out, _ = run(Q, I)
    return out


# revision 7
# speedup vs baseline: 1.7521x; 1.0079x over previous
"""Bilateral filter (nn_BilateralFilter) Trainium2 Bass kernel, v2.

Reference semantics (KERNEL_SIZE=5, THETA_ALPHA=2.0, THETA_BETA=0.1):
    w_k   = exp(-(dx^2+dy^2)/8)                      (24 offsets, center dropped)
    Ki    = exp(-50*(I(p+k) - I(p))^2)               per image channel c
    out[c,n,p] = sum_k w_k*Ki[c,k,p]*Q(n,p+k) / sum_k w_k*Ki[c,k,p]

v2 strategy (vs v1's all-DVE fold at 199us):
  The range kernel is symmetric in its two endpoints, so
  Ki_k(p) = Ki_{-k}(p+k) and therefore
      Qtilde(p) = sum_j U_j(p - j),   U_j(q) = w_j*Ki_j(q)*Q(q)
  i.e. every product is SAME-SITE (no shifted Q reads), and the 24-tap
  shifted fold becomes 24 one-hot-shift matmuls accumulated in fp32 PSUM
  on the otherwise-idle TensorEngine.  Per core: partitions = (c=3, y=42)
  over two U-row groups [-2,40) and [40,82); row shifts live in the
  host-precomputed 0/1 stationaries (with 2-row cross-group halo
  matmuls), column shifts are free-dim offsets of the moving operand.
  DVE keeps only subs, the 24 U-products (fp16, 2x mode), reciprocal and
  the final PSUM*1/norm; ACT does Square and per-slot Exp with the full
  spatial weight folded into the exp bias (+SHIFT trick cancels in the
  division).  norm(p) = sum_k w_k Ki_k(p) rides the same PSUM path with
  unshifted stationaries.
Sharding: 8 cores = 2 batches x 4 row-slabs of 80 output rows.
"""

import numpy as np
import ml_dtypes

B, C, NCL = 2, 3, 6
H = W = 320
KS, PAD = 5, 2
NSLAB = 4
R = H // NSLAB            # 80 output rows per shard
COEF = 50.0               # 1/(2*theta_beta^2)
SHIFT = 8.0               # exponent shift, cancels in the division
YG = 42                   # U-rows per group ([-2,40) and [40,82))
NP_ = 3 * YG              # 126 partitions for U/kw tiles
MP = 3 * 40               # 120 output partitions (c, y40)
XI = W + 8                # 328: I cols incl +-4 pad
XU = W + 4                # 324: U/kw cols incl +-2 pad
IR = R + 8                # 88 I rows per core
QR = R + 4                # 84 Q rows per core
NST = 14                  # stationaries

_CACHE: dict = {}


def _stationaries():
    """14 one-hot stationaries [126, 14*120] fp16, built on host.

    Slot map (dr = raw 0..4, dr' = dr-2):
      0..4  : S_A_dr   main grp A: p=(c, py=y+2-dr'), m=(c,y), py<42
      5..6  : S_Ah_dr  halo  grp A (dr=0,1): p=(c, pyB), U^B rows 40,41
      7..11 : S_B_dr   main grp B: p=(c, py=my-dr'),  m=(c,my), py>=0
      12..13: S_Bh_dr  halo  grp B (dr=3,4): p=(c, pyA), U^A rows 38,39
    """
    s = np.zeros((NP_, NST * MP), np.float32)

    def S(idx):
        return s[:, idx * MP:(idx + 1) * MP].reshape(NP_, 3, 40)

    for dr in range(5):
        drp = dr - 2
        sa = S(dr)
        sb = S(7 + dr)
        for c in range(3):
            for y in range(40):
                py = y + 2 - drp
                if 0 <= py < YG:
                    sa[c * YG + py, c, y] = 1.0
                py = y - drp
                if 0 <= py < YG:
                    sb[c * YG + py, c, y] = 1.0
    # grp A halo: U rows q=40,41 live in tile B partitions 0,1
    for i, dr in enumerate((0, 1)):
        sh = S(5 + i)
        for c in range(3):
            for pyb in range(2):
                y = 40 + pyb + dr - 2
                if 0 <= y < 40:
                    sh[c * YG + pyb, c, y] = 1.0
    # grp B halo: U rows q=38,39 live in tile A partitions 40,41
    for i, dr in enumerate((3, 4)):
        sh = S(12 + i)
        for c in range(3):
            for pya in (40, 41):
                q = pya - 2
                my = q + (dr - 2) - 40
                if 0 <= my < 40:
                    sh[c * YG + pya, c, my] = 1.0
    return s.astype(ml_dtypes.bfloat16)


def _emit(tc, i_ap, q_ap, s_ap, out_ap):
    """i_ap: (264, 328) fp16   rows = (c, 88)
    q_ap: (84, 1944) fp16     rows = U-rows [-2,82), cols (n,324)
    s_ap: (126, 1680) fp16    stationaries
    out_ap: (80, 5760) fp16   cols (c, n, x)
    """
    import concourse.bass as bass
    import concourse.mybir as mybir

    f16 = mybir.dt.float16
    bf16 = mybir.dt.bfloat16
    f32 = mybir.dt.float32
    AF = mybir.ActivationFunctionType
    nc = tc.nc
    SLOTS = [(dr, dc) for dr in range(5) for dc in range(5)
             if not (dr == 2 and dc == 2)]
    BIAS = {
        (dr, dc): SHIFT - ((dr - 2) ** 2 + (dc - 2) ** 2) / 8.0
        for dr, dc in SLOTS
    }

    with tc.tile_pool(name="p", bufs=1) as pool, \
            tc.psum_pool(name="pp", bufs=1) as ppool:
        stat = pool.tile([NP_, NST * MP], bf16, tag="stat")
        nc.sync.dma_start(stat[:, :], s_ap[:, :])

        distinct = sorted(set(BIAS.values()))
        bcol = {v: j for j, v in enumerate(distinct)}
        bias_t = pool.tile([NP_, len(distinct)], mybir.dt.float32, tag="bias")
        for v, j in bcol.items():
            nc.vector.memset(bias_t[:, j:j + 1], v)

        def st(idx):
            return stat[:, idx * MP:(idx + 1) * MP]

        # per-(grp, dr) shifted I copies [(c,42), 328]
        Idr = {}
        for g in range(2):
            for dr in range(5):
                t = pool.tile([NP_, XI], f16, tag=f"I{g}{dr}")
                for c in range(3):
                    nc.sync.dma_start(
                        t[c * YG:(c + 1) * YG, :],
                        i_ap[c * IR + g * YG + dr: c * IR + g * YG + dr + YG, :],
                    )
                Idr[(g, dr)] = t
        # Q replicated over c: [(c,42), (n,324)] per grp
        Qrep = []
        for g in range(2):
            t = pool.tile([NP_, NCL * XU], f16, tag=f"Q{g}")
            for c in range(3):
                nc.scalar.dma_start(
                    t[c * YG:(c + 1) * YG, :],
                    q_ap[g * YG:(g + 1) * YG, :],
                )
            Qrep.append(t)

        # kw[(c,42), (slot25, 324)]: d -> square -> exp(in place)
        kw = []
        for g in range(2):
            kwg = pool.tile([NP_, 25 * XU], bf16, tag=f"kw{g}")
            d = pool.tile([NP_, 5 * XU], f16, tag=f"d{g}", bufs=2)
            for dr in range(5):
                # d[dc, x] = I(q + (dr', dc')) - I(q); all 5 dc in one op
                in0 = bass.AP(
                    tensor=Idr[(g, dr)].tensor, offset=Idr[(g, dr)].offset,
                    ap=[[XI, NP_], [1, 5], [1, XU]],
                )
                in1 = bass.AP(
                    tensor=Idr[(g, 2)].tensor, offset=Idr[(g, 2)].offset + 2,
                    ap=[[XI, NP_], [0, 5], [1, XU]],
                )
                dst = d[:, :].rearrange("p (dc x) -> p dc x", dc=5)
                nc.vector.tensor_sub(dst, in0, in1)
                nc.scalar.activation(
                    kwg[:, dr * 5 * XU:(dr + 1) * 5 * XU], d[:, :], AF.Square
                )
            for dr, dc in SLOTS:
                sl = dr * 5 + dc
                nc.scalar.activation(
                    kwg[:, sl * XU:(sl + 1) * XU],
                    kwg[:, sl * XU:(sl + 1) * XU],
                    AF.Exp, bias=bias_t[:, bcol[BIAS[(dr, dc)]]:
                                        bcol[BIAS[(dr, dc)]] + 1],
                    scale=-COEF,
                )
            kw.append(kwg)

        # PSUM: per grp 4 fbanks of (n6, x80) + 1 norm bank
        NFB = 4
        XB = W // NFB  # 80
        qt = [ppool.tile([MP, NCL * XB], f32, tag=f"qt{b}", name=f"qt{b}")
              for b in range(NFB)]
        nrm = ppool.tile([MP, W], f32, tag="nrm", name="nrm")

        def u_prod(g, dr, dc, tag, bufs=1):
            sl = dr * 5 + dc
            u = pool.tile([NP_, NCL * XU], bf16, tag=tag, bufs=bufs)
            kws = bass.AP(
                tensor=kw[g].tensor, offset=kw[g].offset + sl * XU,
                ap=[[25 * XU, NP_], [0, NCL], [1, XU]],
            )
            qs = bass.AP(
                tensor=Qrep[g].tensor, offset=Qrep[g].offset,
                ap=[[NCL * XU, NP_], [XU, NCL], [1, XU]],
            )
            nc.vector.tensor_mul(
                u[:, :].rearrange("p (n x) -> p n x", n=NCL), kws, qs
            )
            return u

        def mm_u(g, u, dr, dc, sidx, start, stop):
            for b in range(NFB):
                mv = bass.AP(
                    tensor=u.tensor, offset=u.offset + (4 - dc) + b * XB,
                    ap=[[NCL * XU, NP_], [XU, NCL], [1, XB]],
                )
                nc.tensor.matmul(
                    qt[b][:, :], st(sidx), mv,
                    start=start, stop=stop,
                )

        def mm_norm(g, dr, dc, start, stop):
            sl = dr * 5 + dc
            mv = bass.AP(
                tensor=kw[g].tensor, offset=kw[g].offset + sl * XU + 2,
                ap=[[25 * XU, NP_], [1, W]],
            )
            nc.tensor.matmul(
                nrm[:, :], st(2 if g == 0 else 9), mv,
                start=start, stop=stop,
            )

        # ---- grp A: norm MMs first (only need kw), then main ----
        for i, (dr, dc) in enumerate(SLOTS):
            mm_norm(0, dr, dc, start=i == 0, stop=i == len(SLOTS) - 1)
        upinA = {}
        first = True
        for dr in range(5):
            for dc in range(5):
                if (dr, dc) == (2, 2):
                    continue
                if dr >= 3:
                    u = u_prod(0, dr, dc, tag=f"up{dr}{dc}")
                    upinA[(dr, dc)] = u
                else:
                    u = u_prod(0, dr, dc, tag="urot", bufs=8)
                mm_u(0, u, dr, dc, dr, start=first, stop=False)
                first = False
        # ---- grp B products for dr 0,1 feed grp A halo ----
        uB01 = {}
        for dr in range(2):
            for dc in range(5):
                u = u_prod(1, dr, dc, tag=f"uB{dr}{dc}")
                uB01[(dr, dc)] = u
                last = (dr, dc) == (1, 4)
                mm_u(0, u, dr, dc, 5 + dr, start=False, stop=last)

        # ---- grp A finals: rnorm + PSUM*rnorm -> out ----
        def finals(g):
            rn = pool.tile([MP, W], f32, tag=f"rn{g}")
            nc.vector.reciprocal_approx_fast(rn[:, :], nrm[:, :])
            ot = pool.tile([MP, NCL * W], f16, tag=f"ot{g}")
            for b in range(NFB):
                dst = bass.AP(
                    tensor=ot.tensor, offset=ot.offset + b * XB,
                    ap=[[NCL * W, MP], [W, NCL], [1, XB]],
                )
                rnb = bass.AP(
                    tensor=rn.tensor, offset=rn.offset + b * XB,
                    ap=[[W, MP], [0, NCL], [1, XB]],
                )
                src = qt[b][:, :].rearrange("p (n x) -> p n x", n=NCL)
                nc.vector.tensor_mul(dst, src, rnb)
            for c in range(3):
                nc.sync.dma_start(
                    out_ap[g * 40:(g + 1) * 40,
                           c * NCL * W:(c + 1) * NCL * W],
                    ot[c * 40:(c + 1) * 40, :],
                )

        finals(0)

        # ---- grp B: norm MMs, then main; halo from pinned U^A ----
        for i, (dr, dc) in enumerate(SLOTS):
            mm_norm(1, dr, dc, start=i == 0, stop=i == len(SLOTS) - 1)
        first = True
        for dr in range(5):
            for dc in range(5):
                if (dr, dc) == (2, 2):
                    continue
                if dr < 2:
                    u = uB01[(dr, dc)]
                else:
                    u = u_prod(1, dr, dc, tag="urot", bufs=8)
                mm_u(1, u, dr, dc, 7 + dr, start=first, stop=False)
                first = False
        for i, dr in enumerate((3, 4)):
            for dc in range(5):
                last = (dr, dc) == (4, 4)
                mm_u(1, upinA[(dr, dc)], dr, dc, 12 + i,
                     start=False, stop=last)

        finals(1)


def _build_program():
    import concourse.bacc as bacc
    import concourse.mybir as mybir
    from concourse import tile

    f16 = mybir.dt.float16

    nc = bacc.Bacc("TRN2", num_devices=8, debug=False)
    I_in = nc.dram_tensor("i_in", [3 * IR, XI], f16, kind="ExternalInput")
    Q_in = nc.dram_tensor("q_in", [QR, NCL * XU], f16, kind="ExternalInput")
    S_in = nc.dram_tensor("s_in", [NP_, NST * MP], mybir.dt.bfloat16,
                          kind="ExternalInput")
    OUT = nc.dram_tensor("out", [R, C * NCL * W], f16, kind="ExternalOutput")

    with tile.TileContext(nc) as tc:
        _emit(tc, I_in.ap(), Q_in.ap(), S_in.ap(), OUT.ap())

    nc.compile()
    return nc


def _get_program():
    if "nc" not in _CACHE:
        _CACHE["nc"] = _build_program()
    return _CACHE["nc"]


def _shard_inputs(Q, I):
    """Host prep: pad, cast fp16, per-shard layouts (see _emit)."""
    Qp = np.pad(
        np.asarray(Q, np.float32), ((0, 0), (0, 0), (PAD, PAD), (PAD, PAD))
    ).astype(np.float16)
    Ip = np.pad(
        np.asarray(I, np.float32), ((0, 0), (0, 0), (4, 4), (4, 4))
    ).astype(np.float16)
    s_np = _stationaries()
    in_maps = []
    for b in range(B):
        for s in range(NSLAB):
            r0 = s * R
            i_sh = Ip[b, :, r0:r0 + IR, :]            # (C, 88, 328)
            q_sh = Qp[b, :, r0:r0 + QR, :]            # (NCL, 84, 324)
            in_maps.append(
                {
                    "i_in": np.ascontiguousarray(i_sh.reshape(3 * IR, XI)),
                    "q_in": np.ascontiguousarray(
                        q_sh.transpose(1, 0, 2).reshape(QR, NCL * XU)
                    ),
                    "s_in": s_np,
                }
            )
    return in_maps


def _assemble(outs):
    # outs: list of 8 arrays (R, C*NCL*W), core order = (b, slab)
    o = np.stack([np.asarray(x) for x in outs]).astype(np.float32)
    o = o.reshape(B, NSLAB, R, C, NCL, W)
    o = o.transpose(0, 3, 4, 1, 2, 5).reshape(B, C, NCL, H, W)
    return o


def run(Q, I, trace=False):
    from concourse.bass_utils import run_bass_kernel_spmd

    nc = _get_program()
    in_maps = _shard_inputs(Q, I)
    res = run_bass_kernel_spmd(nc, in_maps, list(range(8)), trace=trace)
    out = _assemble([res.results[i]["out"] for i in range(8)])
    return out, res


def kernel(Q, I):
    out, _ = run(Q, I)
    return out
